# revision 2
# baseline (speedup 1.0000x reference)
"""Distributed Trainium2 kernel for nn_Attn_77970836292156 (v3).

Cross-attention: fused QKV proj + per-head RMSNorm + RoPE + bf16 SDPA
(4096 keys) + output projection; tensor-parallel on heads (2 heads/core).

v3 structure (vs v2):
- Phase 2 runs as 4 q-quarters (512 q each). Scores st tiles span a
  kc-PAIR [128 keys x 1024(=2 kc x 512 q)] with 3 PSUM bufs, giving the
  software pipeline distance TWO units: every PE semaphore wait is
  pre-satisfied ~1 unit early, so the PE never couples just-in-time to
  the ACT exp stream (which held the whole phase at mid p-state in v2).
- rs_k is multiplied into kTn during phase 1 (broadcast via DRAM bounce)
  so the exp uses a constant scale and can span a full [128,1024] tile.
- Each quarter's oT slice is AllGathered (Shared output) right after its
  divide; AG i overlaps quarter i+1's compute. The column-sharded W_out
  (each core computes out[:, c*128:(c+1)*128] for all q from the full
  gathered oT) keeps the program SPMD-identical.
- Phase-1 elementwise work split across engines: PSUM evicts on DVE,
  SBUF-only rope/square chains on GpSimd, rs Ln/Exp on ACT.
"""

import os

import numpy as np
import ml_dtypes

import concourse.bass as bass
import concourse.tile as tile
from concourse import bacc, mybir
from concourse.bass_utils import run_bass_kernel_spmd

BF16 = mybir.dt.bfloat16
F32 = mybir.dt.float32

N = 2048
M = 2048
NK = N + M
D = 1024
H = 16
DH = 64
HL = 2
DL = HL * DH
F = 1024
P = 128
NCORES = 8
EPS = 1e-6
ROPE_BASE = 10000.0
SCALE = 0.125
QW = 512                 # q-quarter width
NQ = N // QW             # 4 quarters
NKP = NK // P            # 32 key chunks
NU = (NKP // 2) * HL     # 32 units per quarter (kc-pair x head)

LAST_RESULT = None


def build_nc():
    nc = bacc.Bacc()

    xT = nc.declare_dram_parameter("xT", [F, N], BF16, isOutput=False)
    yT = nc.declare_dram_parameter("yT", [F, M], BF16, isOutput=False)
    wq = nc.declare_dram_parameter("wq", [F, DL], BF16, isOutput=False)
    wk = nc.declare_dram_parameter("wk", [F, DL], BF16, isOutput=False)
    wv = nc.declare_dram_parameter("wv", [F, DL], BF16, isOutput=False)
    wck = nc.declare_dram_parameter("wck", [F, DL], BF16, isOutput=False)
    wcv = nc.declare_dram_parameter("wcv", [F, DL], BF16, isOutput=False)
    woc = nc.declare_dram_parameter("woc", [D, P], BF16, isOutput=False)
    boc = nc.declare_dram_parameter("boc", [P, 1], F32, isOutput=False)
    sq = nc.declare_dram_parameter("sq", [P, N], BF16, isOutput=False)
    ckc = nc.declare_dram_parameter("ckc", [P, NK], BF16, isOutput=False)
    cks = nc.declare_dram_parameter("cks", [P, NK], BF16, isOutput=False)
    hmq = nc.declare_dram_parameter("hmq", [P, HL], BF16, isOutput=False)
    hmk = nc.declare_dram_parameter("hmk", [P, HL], BF16, isOutput=False)
    hmc = nc.declare_dram_parameter("hmc", [P, HL], BF16, isOutput=False)
    out_ext = nc.declare_dram_parameter("out", [P, N], F32, isOutput=True)

    ag_in = nc.dram_tensor("ag_in", [NQ, P, QW], BF16)
    ag_out = nc.dram_tensor("ag_out", [NQ, NCORES * P, QW], BF16,
                            addr_space="Shared")
    rs_dram = nc.dram_tensor("rs_dram", [HL, NK + N], BF16)
    rd_dram = nc.dram_tensor("rd_dram", [2, 1, QW], F32)

    with tile.TileContext(nc) as tc, \
            tc.tile_pool(name="singles", bufs=1) as singles:

        # ---------------- input DMAs, priority order ----------------
        def load_w(param):
            t = singles.tile([P, 8, DL], BF16, tag=param.name + "_sb")
            for f in range(8):
                nc.sync.dma_start(out=t[:, f, :],
                                  in_=param[f * P:(f + 1) * P, :])
            return t

        wq_sb, wk_sb = load_w(wq), load_w(wk)

        xT_sb = singles.tile([P, 8, N], BF16)
        for f in range(8):
            nc.sync.dma_start(out=xT_sb[:, f, :], in_=xT[f * P:(f + 1) * P, :])

        sq_sb = singles.tile([P, N], BF16)
        ckc_sb = singles.tile([P, NK], BF16)
        cks_sb = singles.tile([P, NK], BF16)
        nc.sync.dma_start(out=ckc_sb, in_=ckc[:, :])
        nc.sync.dma_start(out=sq_sb, in_=sq[:, :])
        nc.sync.dma_start(out=cks_sb, in_=cks[:, :])

        hmq_sb = singles.tile([P, HL], BF16)
        hmk_sb = singles.tile([P, HL], BF16)
        hmc_sb = singles.tile([P, HL], BF16)
        nc.sync.dma_start(out=hmq_sb, in_=hmq[:, :])
        nc.sync.dma_start(out=hmk_sb, in_=hmk[:, :])
        nc.sync.dma_start(out=hmc_sb, in_=hmc[:, :])

        wv_sb, wck_sb, wcv_sb = load_w(wv), load_w(wck), load_w(wcv)

        yT_sb = singles.tile([P, 8, M], BF16)
        for f in range(8):
            nc.sync.dma_start(out=yT_sb[:, f, :], in_=yT[f * P:(f + 1) * P, :])

        woc_sb = singles.tile([P, 8, P], BF16)
        for f in range(8):
            nc.sync.dma_start(out=woc_sb[:, f, :],
                              in_=woc[f * P:(f + 1) * P, :])
        boc_sb = singles.tile([P, 1], F32)
        nc.sync.dma_start(out=boc_sb, in_=boc[:, :])

        onesb = singles.tile([P, 512], BF16)
        nc.vector.memset(onesb, 1.0)
        eps2 = singles.tile([HL, 1], F32)
        nc.vector.memset(eps2, EPS)

        qTn = singles.tile([P, N], BF16)
        kTn = singles.tile([P, NK], BF16)
        v_all = singles.tile([P, NKP, 130], BF16)
        nc.gpsimd.memset(v_all, 1.0)
        oT = singles.tile([P, N], BF16)
        scr_a = singles.tile([HL, N], F32)
        scr_b = singles.tile([HL, N], F32)
        rs_bf = singles.tile([HL, N], BF16)
        t1q = singles.tile([P, N], BF16)
        t1k = singles.tile([P, N], BF16)

        # ============ phase 1a: q+k joint projection, f-outer ============
        with tc.tile_pool(name="qk_ps", bufs=1, space="PSUM") as qk_ps:
            psq = [qk_ps.tile([P, 512], F32, name=f"psq{t}", tag=f"psq{t}")
                   for t in range(4)]
            psk = [qk_ps.tile([P, 512], F32, name=f"psk{t}", tag=f"psk{t}")
                   for t in range(4)]
            for f in range(8):
                for t in range(4):
                    nc.tensor.matmul(psq[t], wq_sb[:, f, :],
                                     xT_sb[:, f, t * 512:(t + 1) * 512],
                                     start=(f == 0), stop=(f == 7))
                for t in range(4):
                    nc.tensor.matmul(psk[t], wk_sb[:, f, :],
                                     xT_sb[:, f, t * 512:(t + 1) * 512],
                                     start=(f == 0), stop=(f == 7))
            for t in range(4):
                cs = slice(t * 512, (t + 1) * 512)
                nc.vector.tensor_mul(t1q[:, cs], psq[t], onesb)
                nc.vector.tensor_mul(t1k[:, cs], psk[t], onesb)

        # ============ phase 1b ============
        with tc.tile_pool(name="ssq_ps", bufs=2, space="PSUM") as ssq_ps, \
                tc.tile_pool(name="vps", bufs=2, space="PSUM") as vps, \
                tc.tile_pool(name="ck_ps", bufs=1, space="PSUM") as ck_ps, \
                tc.tile_pool(name="p1w", bufs=2) as p1w, \
                tc.tile_pool(name="bc", bufs=1) as bc, \
                tc.tile_pool(name="rope", bufs=1) as rope:

            def ssq_chunks(t1, hm_sb):
                # square on gpsimd (SBUF only), mask-matmul on PE, evict DVE
                for t in range(4):
                    cs = slice(t * 512, (t + 1) * 512)
                    qsq = p1w.tile([P, 512], BF16, name="qsq", tag="qsq")
                    nc.gpsimd.tensor_mul(qsq, t1[:, cs], t1[:, cs])
                    ssq = ssq_ps.tile([HL, 512], F32, name="ssq", tag="ssq")
                    nc.tensor.matmul(ssq, hm_sb, qsq, start=True, stop=True)
                    nc.vector.tensor_mul(scr_a[:, cs], ssq, onesb[0:HL, :])

            def rs_compute(dram_off, fold_scale):
                nc.scalar.activation(out=scr_b, in_=scr_a,
                                     func=mybir.ActivationFunctionType.Ln,
                                     bias=eps2)
                kw = dict(bias=lnscale2) if fold_scale else {}
                nc.scalar.activation(out=rs_bf, in_=scr_b,
                                     func=mybir.ActivationFunctionType.Exp,
                                     scale=-0.5, **kw)
                nc.sync.dma_start(out=rs_dram[:, dram_off:dram_off + N],
                                  in_=rs_bf)

            def bcast_rs(dram_off, npos, tag):
                rb = bc.tile([P, npos], BF16, name=tag, tag=tag,
                             padded_shape=[P, N])
                for h in range(HL):
                    hap = rs_dram[h:h + 1, dram_off:dram_off + npos]
                    bsrc = bass.AP(tensor=hap.tensor, offset=hap.offset,
                                   ap=[[0, DH]] + hap.ap[1:])
                    nc.sync.dma_start(out=rb[h * DH:(h + 1) * DH, :],
                                      in_=bsrc)
                return rb

            lnscale2 = singles.tile([HL, 1], F32)
            nc.vector.memset(lnscale2, float(np.log(SCALE)))

            def rope_apply(t1, npos, c_sb, s_sb, tab_off, dst, dst_off,
                           mul_rs):
                tab = slice(tab_off, tab_off + npos)
                sl = slice(dst_off, dst_off + npos)
                m1 = rope.tile([P, npos], BF16, name="m1", tag="m1",
                               padded_shape=[P, N])
                nc.gpsimd.tensor_mul(m1, t1[:, 0:npos], c_sb[:, tab])
                t1r = rope.tile([P, npos], BF16, name="t1r", tag="t1r",
                                padded_shape=[P, N])
                for h in range(HL):
                    b = h * DH
                    nc.sync.dma_start(out=t1r[b:b + 32, :],
                                      in_=t1[b + 32:b + 64, 0:npos])
                    nc.sync.dma_start(out=t1r[b + 32:b + 64, :],
                                      in_=t1[b:b + 32, 0:npos])
                r1 = rope.tile([P, npos], BF16, name="r1", tag="r1",
                               padded_shape=[P, N])
                nc.gpsimd.tensor_mul(r1, t1r, s_sb[:, tab])
                s2 = rope.tile([P, npos], BF16, name="s2", tag="s2",
                               padded_shape=[P, N])
                nc.gpsimd.tensor_add(s2, m1, r1)
                nc.gpsimd.tensor_mul(dst[:, sl], s2, mul_rs)

            # ---- q ----
            ssq_chunks(t1q, hmq_sb)
            rs_compute(NK, False)
            rsb = bcast_rs(NK, N, "rsb")
            rope_apply(t1q, N, ckc_sb, sq_sb, 0, qTn, 0, rsb)

            # ---- k(self) ----
            ssq_chunks(t1k, hmk_sb)
            rs_compute(0, False)
            rkb = bcast_rs(0, N, "rkb")
            rope_apply(t1k, N, ckc_sb, cks_sb, 0, kTn, 0, rkb)

            # ---- v(self) ----
            def v_chunk(t, src_sb, w_sb, tt):
                ps = vps.tile([P, DL], F32, name="vps", tag="vps")
                for f in range(8):
                    nc.tensor.matmul(ps, src_sb[:, f, tt * P:(tt + 1) * P],
                                     w_sb[:, f, :], start=(f == 0),
                                     stop=(f == 7))
                dstap = v_all[:, t, :]
                dst = bass.AP(tensor=dstap.tensor, offset=dstap.offset,
                              ap=[dstap.ap[0], [65, 2], [1, DH]])
                srcap = ps[:, :]
                src = bass.AP(tensor=srcap.tensor, offset=srcap.offset,
                              ap=[srcap.ap[0], [DH, 2], [1, DH]])
                ob = onesb[:, :]
                one2 = bass.AP(tensor=ob.tensor, offset=ob.offset,
                               ap=[ob.ap[0], [0, 2], [1, DH]])
                nc.vector.tensor_mul(dst, src, one2)

            for t in range(N // P):
                v_chunk(t, xT_sb, wv_sb, t)

            # ---- ck ----
            t1c = t1k  # reuse (k rope complete before ck evicts overwrite)
            psc = [ck_ps.tile([P, 512], F32, name=f"psc{t}", tag=f"psc{t}")
                   for t in range(4)]
            for f in range(8):
                for t in range(4):
                    nc.tensor.matmul(psc[t], wck_sb[:, f, :],
                                     yT_sb[:, f, t * 512:(t + 1) * 512],
                                     start=(f == 0), stop=(f == 7))
            for t in range(4):
                cs = slice(t * 512, (t + 1) * 512)
                nc.vector.tensor_mul(t1c[:, cs], psc[t], onesb)
            ssq_chunks(t1c, hmc_sb)
            rs_compute(N, False)
            rcb = bcast_rs(N, M, "rkb")   # reuse rkb buffer (dep-ordered)
            rope_apply(t1c, M, ckc_sb, cks_sb, N, kTn, N, rcb)

            # ---- cv ----
            for t in range(M // P):
                v_chunk(N // P + t, yT_sb, wcv_sb, t)

        # ============ phase 2: attention quarters + AG ============
        def attn_quarter(qq, st_ps, pv_ps, p2w, p2s):
            q0 = qq * QW
            pv = [pv_ps.tile([65, QW], F32, name=f"pv{qq}{h}", tag=f"pv{h}")
                  for h in range(HL)]

            def st_mm(u):
                kcp, h = u // HL, u % HL
                hs = slice(h * DH, (h + 1) * DH)
                st = st_ps.tile([P, 1024], F32, name=f"st{qq}_{u}", tag="st")
                for half in range(2):
                    kc = kcp * 2 + half
                    nc.tensor.matmul(
                        st[:, half * QW:(half + 1) * QW],
                        kTn[hs, kc * P:(kc + 1) * P],
                        qTn[hs, q0:q0 + QW],
                        start=True, stop=True)
                return st

            def exp_u(u, st):
                es = p2w.tile([P, 1024], BF16, name=f"es{qq}_{u}", tag="es")
                nc.scalar.activation(out=es, in_=st,
                                     func=mybir.ActivationFunctionType.Exp,
                                     scale=SCALE)
                return es

            def pv_mm(u, es):
                kcp, h = u // HL, u % HL
                for half in range(2):
                    kc = kcp * 2 + half
                    nc.tensor.matmul(
                        pv[h][:, :],
                        v_all[:, kc, h * 65:(h + 1) * 65],
                        es[:, half * QW:(half + 1) * QW],
                        start=(kc == 0), stop=(kc == NKP - 1))

            sts = {0: st_mm(0), 1: st_mm(1)}
            ess = {}
            for u in range(NU):
                ess[u] = exp_u(u, sts[u])
                if u + 2 < NU:
                    sts[u + 2] = st_mm(u + 2)
                pv_mm(u, ess[u])
                del sts[u]

            for h in range(HL):
                den = p2s.tile([1, QW], F32, name="den", tag="den")
                nc.vector.tensor_mul(den, pv[h][64:65, :], onesb[0:1, :])
                rd = p2s.tile([1, QW], F32, name="rd", tag="rd")
                nc.vector.reciprocal_approx_fast(rd, den)
                nc.sync.dma_start(out=rd_dram[h, :, :], in_=rd)
                rdb = p2s.tile([DH, QW], F32, name="rdb", tag="rdb")
                hap = rd_dram[h, 0:1, :]
                bsrc = bass.AP(tensor=hap.tensor, offset=hap.offset,
                               ap=[[0, DH]] + hap.ap[1:])
                nc.sync.dma_start(out=rdb, in_=bsrc)
                nc.vector.tensor_mul(oT[h * DH:(h + 1) * DH, q0:q0 + QW],
                                     pv[h][0:64, :], rdb)

            nc.sync.dma_start(out=ag_in[qq], in_=oT[:, q0:q0 + QW])
            nc.gpsimd.collective_compute(
                "AllGather", mybir.AluOpType.bypass,
                replica_groups=[list(range(NCORES))],
                ins=[ag_in[qq]],
                outs=[ag_out[qq]],
            )

        for qq in range(NQ):
            with tc.tile_pool(name=f"st{qq}", bufs=3, space="PSUM") as st_ps, \
                    tc.tile_pool(name=f"pv{qq}", bufs=1, space="PSUM") as pv_ps, \
                    tc.tile_pool(name=f"p2w{qq}", bufs=4) as p2w, \
                    tc.tile_pool(name=f"p2s{qq}", bufs=2) as p2s:
                attn_quarter(qq, st_ps, pv_ps, p2w, p2s)

        # ============ phase 3: column-sharded out-proj per quarter ========
        with tc.tile_pool(name="z_ps", bufs=2, space="PSUM") as z_ps, \
                tc.tile_pool(name="zw", bufs=2) as zw, \
                tc.tile_pool(name="zout", bufs=2) as zout:
            for qq in range(NQ):
                zp = z_ps.tile([P, QW], F32, name="zp", tag="zp")
                for r in range(2):
                    of_sb = zw.tile([P, 4, QW], BF16, name="of_sb", tag="of")
                    for s in range(4):
                        g = r * 4 + s
                        nc.sync.dma_start(
                            out=of_sb[:, s, :],
                            in_=ag_out[qq, g * P:(g + 1) * P, :])
                    for s in range(4):
                        g = r * 4 + s
                        nc.tensor.matmul(zp, woc_sb[:, g, :], of_sb[:, s, :],
                                         start=(g == 0),
                                         stop=(g == NCORES - 1))
                zs = zout.tile([P, QW], F32, name="zs", tag="zs")
                nc.vector.scalar_tensor_tensor(
                    out=zs, in0=zp, scalar=boc_sb[:, 0:1], in1=onesb,
                    op0=mybir.AluOpType.add, op1=mybir.AluOpType.mult)
                nc.sync.dma_start(out=out_ext[:, qq * QW:(qq + 1) * QW],
                                  in_=zs)
    return nc


def _bf16(a):
    return np.ascontiguousarray(a).astype(ml_dtypes.bfloat16)


def _rope_tables(npos, g_first, g_second, n_first):
    """Cos/sin tables [128, npos]; g enters as the ratio g[j]/g[sigma(j)]
    on the (signed) sin table — g itself is folded into the projection
    weight columns."""
    inv = 1.0 / (ROPE_BASE ** (np.arange(0, DH, 2, dtype=np.float64) / DH))
    pos = np.arange(0, npos, dtype=np.float64)
    ang = pos[:, None] * inv[None, :]
    cos = np.cos(ang).T
    sin = np.sin(ang).T
    C = np.zeros((DH, npos), np.float64)
    S = np.zeros((DH, npos), np.float64)
    g = np.zeros((DH, npos), np.float64)
    g[:, :n_first] = np.asarray(g_first, np.float64)[:, None]
    if n_first < npos:
        g[:, n_first:] = np.asarray(g_second, np.float64)[:, None]
    C[:32] = cos
    C[32:] = cos
    S[:32] = -sin
    S[32:] = sin
    gsig = np.concatenate([g[32:], g[:32]], axis=0)
    with np.errstate(divide="ignore", invalid="ignore"):
        ratio = np.where(gsig != 0, g / np.where(gsig == 0, 1.0, gsig), 0.0)
    S *= ratio
    return _bf16(np.concatenate([C, C], axis=0)), \
        _bf16(np.concatenate([S, S], axis=0))


def _gmask(g):
    g = np.asarray(g, np.float64)
    inv = np.where(g != 0, 1.0 / np.where(g == 0, 1.0, g) ** 2, 0.0) / DH
    hm = np.zeros((P, HL), np.float64)
    for h in range(HL):
        hm[h * DH:(h + 1) * DH, h] = inv
    return _bf16(hm)


_NC_CACHE = None


def kernel(x, y, W_qkv, W_ckv, W_out, b_out, g_q, g_k, g_ck, n_heads):
    global LAST_RESULT, _NC_CACHE
    x = np.asarray(x, np.float32)
    y = np.asarray(y, np.float32)
    W_qkv = np.asarray(W_qkv, np.float32)
    W_ckv = np.asarray(W_ckv, np.float32)
    W_out = np.asarray(W_out, np.float32)
    b_out = np.asarray(b_out, np.float32)
    g_q = np.asarray(g_q, np.float64)
    g_k = np.asarray(g_k, np.float64)
    g_ck = np.asarray(g_ck, np.float64)

    xT = _bf16(x[0].T)
    yT = _bf16(y[0].T)
    Wq, Wk, Wv = (W_qkv[:, i * D:(i + 1) * D] for i in range(3))
    Wck, Wcv = (W_ckv[:, i * D:(i + 1) * D] for i in range(2))
    Wq = Wq * np.tile(g_q, H).astype(np.float32)
    Wk = Wk * np.tile(g_k, H).astype(np.float32)
    Wck = Wck * np.tile(g_ck, H).astype(np.float32)

    _, sqh = _rope_tables(N, g_q, g_q, N)
    ckch, cksh = _rope_tables(NK, g_k, g_ck, N)
    hmqh, hmkh, hmch = _gmask(g_q), _gmask(g_k), _gmask(g_ck)

    in_maps = []
    for c in range(NCORES):
        sl = slice(c * DL, (c + 1) * DL)
        csl = slice(c * P, (c + 1) * P)
        in_maps.append({
            "xT": xT, "yT": yT,
            "wq": _bf16(Wq[:, sl]), "wk": _bf16(Wk[:, sl]),
            "wv": _bf16(Wv[:, sl]), "wck": _bf16(Wck[:, sl]),
            "wcv": _bf16(Wcv[:, sl]),
            "woc": _bf16(W_out[:, csl]),
            "boc": np.ascontiguousarray(
                b_out[csl].reshape(P, 1).astype(np.float32)),
            "sq": sqh, "ckc": ckch, "cks": cksh,
            "hmq": hmqh, "hmk": hmkh, "hmc": hmch,
        })

    if _NC_CACHE is None:
        _NC_CACHE = build_nc()
        if not _NC_CACHE.is_finalized():
            _NC_CACHE.finalize()
    nc = _NC_CACHE

    res = run_bass_kernel_spmd(
        nc, in_maps, core_ids=list(range(NCORES)),
        trace=bool(os.environ.get("BASS_TRACE")),
    )
    LAST_RESULT = res
    out = np.empty((N, D), np.float32)
    for c in range(NCORES):
        zT = np.asarray(res.results[c]["out"], np.float32)
        out[:, c * P:(c + 1) * P] = zT.T
    return out[None, :, :]
